# revision 1
# baseline (speedup 1.0000x reference)
"""Trainium2 Bass kernel for CARE position encoding (rotor sandwich product).

The reference computes out = R x R~ where R is a product of 4 plane rotors
(cos(phi_i) + sin(phi_i) e_mi) with phi_i = 0.5 * c_i * theta[pos, i].
Algebraically this factorizes into 4 sequential Givens-rotation stages: for
plane bivector e_m, the 8 basis blades A with |A & m| == 1 rotate in 4
disjoint pairs (A, A^m) by angle 2*phi with pair signs tau = C[A, m, A^m];
the other 8 blades pass through unchanged:
    out[a] = c2*x[a] + tau*s2*x[b] ;  out[b] = c2*x[b] - tau*s2*x[a]

Implementation (data-parallel across 8 cores, batch-sharded, 2 rows/core):
 - angles once per core: th_i = float(pos)*f_i ; A_i = th_i*c_i  (bit-exact
   match of the reference's effective rotation angle), Cody-Waite range
   reduction mod 2pi, ScalarE Sin for cos table C and a 6-block sin slot
   table SSX = [s,-s,-s,s,s,-s] whose block sequence makes every plane's
   pair-sign pattern an affine function of the access-pattern indices.
 - per x-tile, per plane: 2-3 big strided DVE ops for the cos part (T) and
   the sign-slotted sin part (U) over all 4 pairs at once, then adds T+U
   written in place into the x-tile.  All access patterns respect the DVE
   3-free-dim ISA limit via stride-nesting merges.
 - every plane's index arithmetic is verified symbolically against the
   input Cayley tensor at kernel-build time.
"""
import numpy as np

import concourse.bass as bass
import concourse.tile as tile
from concourse import bacc, mybir
from concourse.bass_utils import run_bass_kernel_spmd

F32 = mybir.dt.float32
F32R = mybir.dt.float32r
I32 = mybir.dt.int32
ALU = mybir.AluOpType

P = 128
NCORES = 8
B, L, MV = 16, 16384, 16
MAX_LEN = 16384
ROWS_PER_CORE = B // NCORES          # 2
N = ROWS_PER_CORE * L                # 32768 positions per core
J = N // P                           # 256 positions per partition
JT = 128                             # positions per partition per x-tile
NT = J // JT

PE_ADDS = False                      # Givens adds on TensorE (float32r)

PLANE_BLADES = (3, 5, 9, 6)

MAGIC = float(np.float32(1.5 * 2 ** 23))
TWO_PI = 2.0 * np.pi
INV_2PI = float(np.float32(1.0 / TWO_PI))
PI_F = float(np.float32(np.pi))
HALF_PI = float(np.float32(np.pi / 2.0))
TWO_PI_F = float(np.float32(TWO_PI))

# sign of each SSX block: SSX = [s2, -s2, -s2, s2, s2, -s2]
SEQ = (1, -1, -1, 1, 1, -1)
NSEQ = len(SEQ)


def _cw_split(val, bits=12):
    def trunc(v):
        u = np.float32(v).view(np.uint32)
        u = np.uint32(u & np.uint32((0xFFFFFFFF << (24 - bits)) & 0xFFFFFFFF))
        return u.view(np.float32)
    c1 = trunc(val)
    c2 = trunc(val - np.float64(c1))
    c3 = np.float32(val - np.float64(c1) - np.float64(c2))
    return float(c1), float(c2), float(c3)


CW1, CW2, CW3 = _cw_split(TWO_PI)

# Per-plane op descriptors.  T-tile layout col = j*8 + (plane-specific comp
# packing, 8 cols per j).  Each sub lists, for the non-j dims:
#   xoff/xdims : component offset / [step,count] dims of the x-tile read
#   toff/tdims : offset / dims in the T/U tile layout (matching iteration)
#   slot_off/slot_steps (U only): SSX block index = slot_off + sum steps*idx
# T reads the forward box, U reads pair-partners (reversed w), ADD writes the
# forward box from T+U.
_PLANE_OPS = {
    3: dict(  # e01: pairs (4q+1, 4q+2)
        tsubs=[dict(xoff=1, xdims=[[4, 4], [1, 2]], toff=0, tdims=[[2, 4], [1, 2]])],
        usubs=[dict(xoff=2, xdims=[[4, 4], [-1, 2]], toff=0, tdims=[[2, 4], [1, 2]],
                    slot_off=0, slot_steps=[0, 1])],
        asubs=[dict(xoff=1, xdims=[[4, 4], [1, 2]], toff=0, tdims=[[2, 4], [1, 2]])],
    ),
    5: dict(  # e02: pairs (8h+2k+1, +3); tau = (-1)^k.  Half-split layout:
        # col = j*8 + w*4 + h*2 + k ; all reads/writes positive-stride.
        tsubs=[dict(xoff=1 + 3 * w, xdims=[[8, 2], [2, 2]],
                    toff=4 * w, tdims=[[2, 2], [1, 2]]) for w in range(2)],
        # U half w reads the partner half (1-w); slot = k + 2w over SEQ
        usubs=[dict(xoff=1 + 3 * (1 - w), xdims=[[8, 2], [2, 2]],
                    toff=4 * w, tdims=[[2, 2], [1, 2]],
                    slot_off=2 * w, slot_steps=[0, 1]) for w in range(2)],
        asubs=[dict(xoff=1 + 3 * w, xdims=[[8, 2], [2, 2]],
                    toff=4 * w, tdims=[[2, 2], [1, 2]]) for w in range(2)],
    ),
    9: dict(  # e03: pairs (2u+1, +7); tau = (+,-,-,+) over u.  Half-split:
        # col = j*8 + w*4 + u ; slot = u + 2w over SEQ.
        tsubs=[dict(xoff=1 + 7 * w, xdims=[[2, 4]],
                    toff=4 * w, tdims=[[1, 4]]) for w in range(2)],
        usubs=[dict(xoff=1 + 7 * (1 - w), xdims=[[2, 4]],
                    toff=4 * w, tdims=[[1, 4]],
                    slot_off=2 * w, slot_steps=[1]) for w in range(2)],
        asubs=[dict(xoff=1 + 7 * w, xdims=[[2, 4]],
                    toff=4 * w, tdims=[[1, 4]]) for w in range(2)],
    ),
    6: dict(  # e12: pairs (8h+2+r, +2); tau=+; T/ADD merge (w,r)->(1,4)
        tsubs=[dict(xoff=2, xdims=[[8, 2], [1, 4]], toff=0, tdims=[[4, 2], [1, 4]])],
        usubs=[dict(xoff=4 + 8 * h, xdims=[[-2, 2], [1, 2]],
                    toff=4 * h, tdims=[[2, 2], [1, 2]],
                    slot_off=0, slot_steps=[1, 0]) for h in range(2)],
        asubs=[dict(xoff=2, xdims=[[8, 2], [1, 4]], toff=0, tdims=[[4, 2], [1, 4]])],
    ),
}


def _iter_idx(dims):
    import itertools
    return itertools.product(*[range(c) for (_, c) in dims])


def _verify_plane_ops(cayley):
    """Symbolically apply the descriptor index arithmetic for one position:
    out[comp] = c2*x[tcomp] + seqsign*s2*x[ucomp], and check it equals the
    Cayley-derived Givens stage for every plane.  Raises on mismatch."""
    for m in PLANE_BLADES:
        ops = _PLANE_OPS[m]
        tmap, umap, usgn, amap = {}, {}, {}, {}
        for sub in ops["tsubs"]:
            for idx in _iter_idx(sub["xdims"]):
                col = sub["toff"] + sum(s * i for (s, _), i in zip(sub["tdims"], idx))
                comp = sub["xoff"] + sum(s * i for (s, _), i in zip(sub["xdims"], idx))
                tmap[col] = comp
        for sub in ops["usubs"]:
            for idx in _iter_idx(sub["xdims"]):
                col = sub["toff"] + sum(s * i for (s, _), i in zip(sub["tdims"], idx))
                comp = sub["xoff"] + sum(s * i for (s, _), i in zip(sub["xdims"], idx))
                blk = sub["slot_off"] + sum(s * i for s, i in
                                            zip(sub["slot_steps"], idx))
                assert 0 <= blk < NSEQ, (m, blk)
                umap[col] = comp
                usgn[col] = SEQ[blk]
        for sub in ops["asubs"]:
            for idx in _iter_idx(sub["xdims"]):
                col = sub["toff"] + sum(s * i for (s, _), i in zip(sub["tdims"], idx))
                comp = sub["xoff"] + sum(s * i for (s, _), i in zip(sub["xdims"], idx))
                amap[col] = comp
        assert sorted(tmap) == sorted(umap) == sorted(amap) == list(range(8)), m
        # ground truth from the Cayley tensor
        for col in range(8):
            a = amap[col]
            assert tmap[col] == a, (m, col, "cos part must read the dst comp")
            b = a ^ m
            assert umap[col] == b, (m, col, umap[col], b)
            # reference: out[a] gets tau*s2*x[b] with tau = C[a, m, b]
            tau = float(cayley[a, m, b])
            assert usgn[col] == tau, (m, col, usgn[col], tau)


def _ap_with_dims(base_ap, extra_off, dims):
    ap = [list(base_ap.ap[0])] + [list(d) for d in dims]
    return bass.AP(base_ap.tensor, base_ap.offset + extra_off, ap)


def _build_program(freqs, coefs):
    nc = bacc.Bacc("TRN2", target_bir_lowering=False, debug=False,
                   enable_asserts=False, num_devices=NCORES)
    x_d = nc.dram_tensor("x", [P, J * MV], F32, kind="ExternalInput")
    pos_d = nc.dram_tensor("pos", [P, J], I32, kind="ExternalInput")
    eye_d = nc.dram_tensor("eye", [P, P], F32, kind="ExternalInput")
    out_d = nc.dram_tensor("out", [P, J * MV], F32, kind="ExternalOutput")

    D = 4 * J  # SSX block stride (elements)
    SIN = mybir.ActivationFunctionType.Sin

    with tile.TileContext(nc) as tc:
        with tc.tile_pool(name="const", bufs=1) as cpool, \
             tc.tile_pool(name="x", bufs=3) as xpool, \
             tc.tile_pool(name="ang", bufs=1) as apool, \
             tc.tile_pool(name="tmp", bufs=3) as tpool, \
             tc.tile_pool(name="ps", bufs=4, space="PSUM") as pspool:

            if PE_ADDS:
                E0 = cpool.tile([P, P], F32)
                nc.sync.dma_start(E0[:], eye_d[:])
                E = cpool.tile([P, P], F32R)
                nc.vector.tensor_copy(E[:], E0[:])

            # ---- once per core: angle tables ----
            Pp = apool.tile([P, J], I32)
            nc.sync.dma_start(Pp[:], pos_d[:])
            posf = apool.tile([P, J], F32)
            nc.vector.tensor_copy(posf[:], Pp[:])

            # Per-plane angle pipelines emitted in rotation order (plane idx 3
            # first) so tile rotations can start as soon as their plane's sin
            # tables are ready instead of waiting for the full 4-plane chain.
            TH = apool.tile([P, 4 * J], F32)
            A = apool.tile([P, 4 * J], F32)
            Q = apool.tile([P, 4 * J], F32)
            Kr = apool.tile([P, 4 * J], F32)
            R = apool.tile([P, 4 * J], F32)
            RC = apool.tile([P, 4 * J], F32)
            C = apool.tile([P, 4 * J], F32)
            SSX = apool.tile([P, NSEQ * D], F32)
            # strided S/SN runs over the SEQ blocks: S {0},{3,4} ; SN {1,2},{5}
            runs = [([0, 1], 1.0), ([3, 2], 1.0), ([1, 2], -1.0), ([5, 1], -1.0)]
            for i in (3, 2, 1, 0):
                sl = slice(i * J, (i + 1) * J)
                nc.vector.tensor_scalar_mul(TH[:, sl], posf[:], float(freqs[i]))
                nc.vector.tensor_scalar_mul(A[:, sl], TH[:, sl], float(coefs[i]))
                nc.vector.tensor_scalar_mul(Q[:, sl], A[:, sl], INV_2PI)
                nc.vector.tensor_scalar(Kr[:, sl], Q[:, sl], MAGIC, MAGIC,
                                        ALU.add, ALU.subtract)
                nc.vector.cody_waite_cascade(R[:, sl], A[:, sl], Kr[:, sl],
                                             CW1, CW2, CW3)
                nc.vector.add_range_wrap(RC[:, sl], R[:, sl],
                                         HALF_PI, PI_F, TWO_PI_F)
                nc.scalar.activation(C[:, sl], RC[:, sl], SIN)
                for (b0, cnt), scale in runs:
                    dst = _ap_with_dims(SSX[:], b0 * D + i * J, [[D, cnt], [1, J]])
                    src = _ap_with_dims(R[:], i * J, [[0, cnt], [1, J]])
                    nc.scalar.activation(dst, src, SIN, scale=scale)

            # ---- x tiles ----
            for t in range(NT):
                X = xpool.tile([P, JT * MV], F32)
                nc.sync.dma_start(X[:], x_d[:, t * JT * MV:(t + 1) * JT * MV])

                for i in (3, 2, 1, 0):
                    m = PLANE_BLADES[i]
                    ops = _PLANE_OPS[m]
                    fd = JT * 8
                    ang0 = i * J + t * JT

                    tu_dt = F32R if PE_ADDS else F32
                    T = tpool.tile([P, fd], tu_dt, tag="t")
                    U = tpool.tile([P, fd], tu_dt, tag="u")

                    for sub in ops["tsubs"]:
                        xr = _ap_with_dims(X[:], sub["xoff"],
                                           [[16, JT]] + sub["xdims"])
                        tw = _ap_with_dims(T[:], sub["toff"],
                                           [[8, JT]] + sub["tdims"])
                        nd = [c for (_, c) in sub["tdims"]]
                        c2b = _ap_with_dims(C[:], ang0,
                                            [[1, JT]] + [[0, c] for c in nd])
                        nc.vector.tensor_mul(tw, xr, c2b)
                    for sub in ops["usubs"]:
                        xr = _ap_with_dims(X[:], sub["xoff"],
                                           [[16, JT]] + sub["xdims"])
                        uw = _ap_with_dims(U[:], sub["toff"],
                                           [[8, JT]] + sub["tdims"])
                        nd = [c for (_, c) in sub["tdims"]]
                        slot = _ap_with_dims(
                            SSX[:], ang0 + sub["slot_off"] * D,
                            [[1, JT]] + [[s * D, c] for s, c in
                                         zip(sub["slot_steps"], nd)])
                        nc.vector.tensor_mul(uw, xr, slot)

                    if PE_ADDS:
                        PS = pspool.tile([P, fd], F32, tag="ps")
                        for h in range(fd // 512):
                            sl = slice(h * 512, (h + 1) * 512)
                            nc.tensor.matmul(PS[:, sl], E[:], T[:, sl],
                                             start=True, stop=False)
                            nc.tensor.matmul(PS[:, sl], E[:], U[:, sl],
                                             start=False, stop=True)
                        for sub in ops["asubs"]:
                            xw = _ap_with_dims(X[:], sub["xoff"],
                                               [[16, JT]] + sub["xdims"])
                            psv = _ap_with_dims(PS[:], sub["toff"],
                                                [[8, JT]] + sub["tdims"])
                            nc.scalar.copy(xw, psv)
                    else:
                        # split the final plane's adds by j-halves so the
                        # output DMA of each half can start early
                        jsplit = 2 if i == 0 else 1
                        jn = JT // jsplit
                        for jh in range(jsplit):
                            for sub in ops["asubs"]:
                                xw = _ap_with_dims(X[:], sub["xoff"] + jh * jn * 16,
                                                   [[16, jn]] + sub["xdims"])
                                tv = _ap_with_dims(T[:], sub["toff"] + jh * jn * 8,
                                                   [[8, jn]] + sub["tdims"])
                                uv = _ap_with_dims(U[:], sub["toff"] + jh * jn * 8,
                                                   [[8, jn]] + sub["tdims"])
                                nc.vector.tensor_add(xw, tv, uv)

                for jh in range(2):
                    hw = JT * MV // 2
                    nc.sync.dma_start(
                        out_d[:, t * JT * MV + jh * hw:
                              t * JT * MV + (jh + 1) * hw],
                        X[:, jh * hw:(jh + 1) * hw])

    nc.compile()
    return nc


_PROGRAM_CACHE = {}


def _get_program(freqs, coefs):
    key = (tuple(freqs), tuple(coefs))
    if key not in _PROGRAM_CACHE:
        _PROGRAM_CACHE[key] = _build_program(freqs, coefs)
    return _PROGRAM_CACHE[key]


def kernel(x, pos, bx, by, bz, bw, theta, cayley, biv_mask, scalar_mask):
    x = np.asarray(x, dtype=np.float32)
    pos = np.asarray(pos)
    theta = np.asarray(theta, dtype=np.float32)
    cayley = np.asarray(cayley, dtype=np.float32)

    assert x.shape == (B, L, MV) and pos.shape == (B, L)

    coefs = [float(np.asarray(c, dtype=np.float32).reshape(MV)[b])
             for c, b in zip((bx, by, bz, bw), PLANE_BLADES)]
    freqs = [float(theta.reshape(MAX_LEN, 4)[1, i]) for i in range(4)]
    th_check = np.arange(MAX_LEN, dtype=np.float32)[:, None] * \
        np.asarray(freqs, dtype=np.float32)[None, :]
    assert np.array_equal(th_check, theta.reshape(MAX_LEN, 4)), \
        "theta table is not linear in position; kernel assumption violated"

    _verify_plane_ops(cayley)

    nc = _get_program(freqs, coefs)

    pos_i = np.clip(pos, 0, MAX_LEN - 1).astype(np.int32)
    eye = np.eye(P, dtype=np.float32)
    in_maps = []
    for g in range(NCORES):
        xg = np.ascontiguousarray(
            x[g * ROWS_PER_CORE:(g + 1) * ROWS_PER_CORE]).reshape(P, J * MV)
        pg = np.ascontiguousarray(
            pos_i[g * ROWS_PER_CORE:(g + 1) * ROWS_PER_CORE]).reshape(P, J)
        in_maps.append({"x": xg, "pos": pg, "eye": eye})

    res = run_bass_kernel_spmd(nc, in_maps, core_ids=list(range(NCORES)))
    out = np.empty((B, L, MV), dtype=np.float32)
    for g in range(NCORES):
        out[g * ROWS_PER_CORE:(g + 1) * ROWS_PER_CORE] = \
            res.results[g]["out"].reshape(ROWS_PER_CORE, L, MV)
    return out

